# revision 23
# baseline (speedup 1.0000x reference)
import math
import numpy as np
import ml_dtypes

N = 50000
F = 64
E = 128
Q = 8
S = 2048
NC = 8
NPC = N // NC          # 6250 clauses per core
NPAD = 6656            # 52 * 128 = 2 * 3328
HALF = 3328            # pair row count: clause nu pairs with nu + HALF
NSC = 13               # super-chunks of 2 128-row chunks
NCH = 52               # 128-clause chunks
SBK = 4                # psum banks of 512 steps each
MHDR = 32              # k2t (f32 bytes) header columns of the mask tensor
ENTROPY_COEF = 0.1

# 2-bit mask packing: byte encodes (m0, m1) of a clause pair; the same byte
# is read twice by the PE, once as fp8e4m3 (value f) and once as fp8e5m2
# (value g).  The quad has zero SECOND difference in both views:
#   f11-f10-f01+f00 = 0  and  g11-g10-g01+g00 = 0
# so two matmul passes recover m0*e0 + m1*e1 exactly; the anchor constant
# (state 00) is cancelled on the host using column sums of the fp8
# stationary tiles, which are DMA'd out.
B00, B10, B01, B11 = 0x0C, 0x17, 0x97, 0x8C
_by = np.array([B00, B10, B01, B11], np.uint8)
_f = _by.view(ml_dtypes.float8_e4m3fn).astype(np.float64)
_g = _by.view(ml_dtypes.float8_e5m2).astype(np.float64)
F00, G00 = _f[0], _g[0]
F10, G10 = _f[1] - _f[0], _g[1] - _g[0]
F01, G01 = _f[2] - _f[0], _g[2] - _g[0]
DET = F10 * G01 - F01 * G10
SA = 32.0              # pass-1 stationary scale (host multiplies by -SA)
SB_ = 1024.0           # pass-2 stationary scale (host multiplies by +SB_)
P1 = -G01 / (DET * SA)   # st1 = P1*e0 + P2*e1      (= 32/27, 8/9)
P2 = G10 / (DET * SA)
Q1 = -F01 / (DET * SB_)  # st2 = Q1*e0 + Q2*e1      (= 14/9, 2/3)
Q2 = F10 / (DET * SB_)

_PROG = None


def _build_prog():
    import sys
    if "/opt/trn_rl_repo" not in sys.path:
        sys.path.insert(0, "/opt/trn_rl_repo")
    from concourse import bass, bacc, tile, mybir

    f32 = mybir.dt.float32
    f16 = mybir.dt.float16
    bf16 = mybir.dt.bfloat16
    f8 = mybir.dt.float8e4
    f8e5 = mybir.dt.float8e5
    AF = mybir.ActivationFunctionType
    ALU = mybir.AluOpType

    nc = bacc.Bacc("TRN2")
    # fv carries [fvT | W1^T] fp8 with a 65th row of [ones | b1]
    fv_d = nc.dram_tensor("fv", [F + 1, NPAD + E], f8, kind="ExternalInput")
    # mask tensor: 32 header cols hold K2T as raw f32 bytes, then the packed
    # mask flattened as [13][2][2048]
    maskT_d = nc.dram_tensor("maskT", [128, MHDR + NSC * 2 * S], f8,
                             kind="ExternalInput")
    stats_d = nc.dram_tensor("stats", [128, SBK * 512 + 2], f16,
                             kind="ExternalOutput")
    xall_d = nc.dram_tensor("xall", [E, NCH, Q], f32, kind="ExternalOutput")

    with tile.TileContext(nc) as tc:
        with (
            tc.tile_pool(name="const", bufs=1) as constp,
            tc.tile_pool(name="big", bufs=1) as bigp,
            tc.tile_pool(name="mask", bufs=1) as maskp,
            tc.tile_pool(name="ps", bufs=1, space=bass.MemorySpace.PSUM) as ps,
        ):
            k2t_sb = constp.tile([E, Q], bf16)
            fsta = constp.tile([E, 1], f8)
            fmov = constp.tile([E, 512], f8)
            bias4 = constp.tile([E, 4], f32)

            fv_sb = bigp.tile([F + 1, NPAD + E], f8)
            ht_sb = bigp.tile([E, NPAD], bf16)
            xall_sb = bigp.tile([E, NCH, Q], f32)
            xs_sb = bigp.tile([E, NCH, Q], f32)
            u1 = bigp.tile([E, 26, Q], f32)
            v1 = bigp.tile([E, 26, Q], f32)
            u2 = bigp.tile([E, 26, Q], f32)
            v2 = bigp.tile([E, 26, Q], f32)
            aE = bigp.tile([E, 26, Q], f32)
            bE = bigp.tile([E, 26, Q], f32)
            aG = bigp.tile([E, 26, Q], f32)
            bG = bigp.tile([E, 26, Q], f32)
            t1 = bigp.tile([E, 26, Q], f32)
            t2 = bigp.tile([E, 26, Q], f32)
            t3 = bigp.tile([E, 26, Q], f32)
            hib = bigp.tile([E, 26, Q], f32)
            st12 = bigp.tile([E, 26, 64], f8)
            stats_sb = bigp.tile([128, SBK * 512 + 2], f16)

            w1_sb = fv_sb[:, NPAD:NPAD + E]

            LP1, LP2 = math.log(P1), math.log(P2)
            LQ1, LQ2 = math.log(Q1), math.log(Q2)

            # constants (vector engine, no deps)
            nc.vector.memset(fsta[:], 1.0)
            nc.vector.memset(fmov[:], 0.0)
            for bi, bv in enumerate((LP1, LP2, LQ1, LQ2)):
                nc.vector.memset(bias4[:, bi:bi + 1], bv)

            nc.sync.dma_start(fv_sb[:], fv_d[:])

            # mask stream on the SP ring in ~1MB blocks (block 0 carries the
            # k2t header)
            MBLK = [(2 * i, 2) for i in range(6)] + [(12, 1)]
            mts = {}
            for bi_, (s0, w) in enumerate(MBLK):
                hdr = MHDR if bi_ == 0 else 0
                mt = maskp.tile([128, hdr + w * 2 * S], f8, tag=f"m{s0}")
                c0 = MHDR + s0 * 2 * S - hdr
                nc.sync.dma_start(mt[:], maskT_d[:, c0:c0 + hdr + w * 2 * S])
                mts[s0] = (w, hdr, mt)

            def mslice(j, i, b):
                for s0, (w, hdr, mt) in mts.items():
                    if s0 <= j < s0 + w:
                        c = hdr + (j - s0) * 2 * S + i * S + 512 * b
                        return mt[:, c:c + 512]

            k2t_f32 = mts[0][2][:, 0:MHDR].bitcast(f32)

            pstat = ps.tile([128, SBK, 512], f32, tag="s", bufs=1, name="s")
            psr = ps.tile([128, 1], f32, tag="r", bufs=1, name="r")
            xps = ps.tile([E, NCH, Q], f32, tag="x", bufs=1, name="x")

            # PE warm-up fillers with K=128 so the HAM activity monitor sees
            # real array utilization and unthrottles before the first matmul.
            for _ in range(9):
                nc.tensor.matmul(pstat[0:1, 0, :], fsta[:], fmov[:],
                                 start=True, stop=True)

            # hT = relu(W1.T @ fvT + b1)  [E, NPAD] bf16; each relu is
            # split in half across ACT and DVE so it never paces the matmuls
            for j in range(NPAD // 512):
                c0 = 512 * j
                ph = ps.tile([E, 512], f32, tag="w", bufs=2, name="w")
                nc.tensor.matmul(ph[:], w1_sb, fv_sb[:, c0:c0 + 512],
                                 start=True, stop=True)
                nc.scalar.activation(ht_sb[:, c0:c0 + 256], ph[:, 0:256],
                                     AF.Relu, bias=0.0, scale=1.0)
                nc.vector.tensor_scalar(ht_sb[:, c0 + 256:c0 + 512],
                                        ph[:, 256:512], 0.0, None, ALU.max)

            # k2t: bf16 copy of the f32 header (after the relus on ACT so it
            # doesn't head-block them while mask block 0 is in flight)
            nc.scalar.activation(k2t_sb[:], k2t_f32, AF.Copy)

            # x'[n,q] = hT_chunk.T @ K2T, pair-interleaved chunk order so the
            # prep of super-chunk k only needs the first 4(k+1) x' matmuls
            for k in range(NSC):
                for i in range(2):
                    c = 2 * k + i
                    nc.tensor.matmul(xps[:, c, :], ht_sb[:, 128 * c:128 * (c + 1)],
                                     k2t_sb[:], start=True, stop=True)
                    c2 = c + 26
                    nc.tensor.matmul(xps[:, c2, :], ht_sb[:, 128 * c2:128 * (c2 + 1)],
                                     k2t_sb[:], start=True, stop=True)

            # bridge fillers: PE stays busy while block-0 prep runs on ACT/DVE
            for _ in range(3):
                nc.tensor.matmul(pstat[0:1, 0, :], fsta[:], fmov[:],
                                 start=True, stop=True)

            first = True
            for (k0, k1) in [(0, 0), (1, 2), (3, 4), (5, 6), (7, 8), (9, 10), (11, 12)]:
                kk = slice(2 * k0, 2 * k1 + 2)
                kx = slice(2 * k0 + 26, 2 * k1 + 2 + 26)
                # x/4 pre-scale keeps every G-stationary fp8 value < 128:
                # values >= 256 (top e4m3 binade) break the mixed-dtype multiply
                nc.scalar.activation(xs_sb[:, kk, :], xps[:, kk, :], AF.Copy,
                                     scale=0.25)
                nc.scalar.activation(xs_sb[:, kx, :], xps[:, kx, :], AF.Copy,
                                     scale=0.25)
                # stationary values: st1 = P1*e0 + P2*e1 (pass1, scale -SA),
                #                    st2 = Q1*e0 + Q2*e1 (pass2, scale +SB_)
                nc.scalar.activation(u1[:, kk, :], xps[:, kk, :], AF.Exp,
                                     bias=bias4[:, 0:1])
                nc.scalar.activation(v1[:, kk, :], xps[:, kx, :], AF.Exp,
                                     bias=bias4[:, 1:2])
                nc.scalar.activation(u2[:, kk, :], xps[:, kk, :], AF.Exp,
                                     bias=bias4[:, 2:3])
                nc.scalar.activation(v2[:, kk, :], xps[:, kx, :], AF.Exp,
                                     bias=bias4[:, 3:4])
                nc.vector.tensor_tensor(aE[:, kk, :], u1[:, kk, :], v1[:, kk, :], ALU.add)
                nc.vector.tensor_tensor(bE[:, kk, :], u2[:, kk, :], v2[:, kk, :], ALU.add)
                nc.gpsimd.tensor_tensor(t1[:, kk, :], xs_sb[:, kk, :], u1[:, kk, :], ALU.mult)
                nc.gpsimd.tensor_tensor(t2[:, kk, :], xs_sb[:, kx, :], v1[:, kk, :], ALU.mult)
                nc.vector.tensor_tensor(aG[:, kk, :], t1[:, kk, :], t2[:, kk, :], ALU.add)
                nc.vector.tensor_tensor(t3[:, kk, :], xs_sb[:, kk, :], u2[:, kk, :], ALU.mult)
                nc.vector.tensor_tensor(t2[:, kk, :], xs_sb[:, kx, :], v2[:, kk, :], ALU.mult)
                nc.vector.tensor_tensor(bG[:, kk, :], t3[:, kk, :], t2[:, kk, :], ALU.add)
                # hi/lo fp8e4m3 splits: hi-cast on GpSimd, then the lo
                # residual via a direct mixed-dtype subtract on DVE
                for srct, cb in ((aE, 0), (aG, 16), (bE, 32), (bG, 48)):
                    dst = st12
                    nc.gpsimd.tensor_copy(dst[:, kk, cb:cb + 8], srct[:, kk, :])
                    nc.vector.tensor_tensor(dst[:, kk, cb + 8:cb + 16],
                                            srct[:, kk, :],
                                            dst[:, kk, cb:cb + 8],
                                            ALU.subtract)

                if first:
                    # x outputs are final after block 0's inputs; ship early
                    nc.scalar.activation(xall_sb[:], xps[:], AF.Copy)
                    nc.scalar.dma_start(xall_d[:], xall_sb[:])
                    first = False

                # stats matmuls: 4 concurrent column groups per (j, bank):
                #   cg0 = (i=0, pass1/e4)   cg1 = (i=0, pass2/e5)
                #   cg2 = (i=1, pass1/e4)   cg3 = (i=1, pass2/e5)
                for j in range(k0, k1 + 1):
                    start = (j == 0)
                    last = (j == NSC - 1)
                    for i in range(2):
                        cg = 64 * i
                        s1k = st12[:, 2 * j + i, 0:32]
                        s2k = st12[:, 2 * j + i, 32:64]
                        for b in range(SBK):
                            nc.tensor.matmul(pstat[cg:cg + 32, b, :], s1k,
                                             mslice(j, i, b),
                                             start=start, stop=last,
                                             skip_group_check=True,
                                             tile_position=(0, cg))
                        for b in range(SBK):
                            nc.tensor.matmul(pstat[cg + 32:cg + 64, b, :], s2k,
                                             mslice(j, i, b).bitcast(f8e5),
                                             start=start, stop=last,
                                             skip_group_check=True,
                                             tile_position=(0, cg + 32))
                        # anchor column: per-column stationary sums via a
                        # ones-column matmul over the merged 64-col tile
                        nc.tensor.matmul(psr[cg:cg + 64, :],
                                         st12[:, 2 * j + i, :], fsta[:],
                                         start=start, stop=last,
                                         skip_group_check=True,
                                         tile_position=(0, cg))
                    if last:
                        nc.scalar.activation(stats_sb[:, 0:512],
                                             pstat[:, 0, :], AF.Copy)
                        nc.vector.tensor_copy(stats_sb[:, 512:1024],
                                              pstat[:, 1, :])
                        nc.sync.dma_start(stats_d[:, 0:1024],
                                          stats_sb[:, 0:1024])
                        nc.scalar.activation(stats_sb[:, 1024:1536],
                                             pstat[:, 2, :], AF.Copy)
                        nc.vector.tensor_copy(stats_sb[:, 1536:2048],
                                              pstat[:, 3, :])
                        nc.vector.tensor_copy(
                            stats_sb[:, 2048:2050].bitcast(f32), psr[:])
                        nc.sync.dma_start(stats_d[:, 1024:2050],
                                          stats_sb[:, 1024:2050])

    nc.finalize()
    return nc


def _get_prog():
    global _PROG
    if _PROG is None:
        _PROG = _build_prog()
    return _PROG


def _prep(feature_vecs, W1, b1, W2, b2, keys, mask):
    m8 = mask.view(np.uint8) if mask.dtype == np.bool_ else mask.astype(np.uint8)
    lut = np.array([B00, B10, B01, B11], np.uint8)

    k2t = (np.asarray(W2, np.float64) @ np.asarray(keys, np.float64).T
           ).astype(np.float32)                       # [E, Q]
    k2t_bytes = np.ascontiguousarray(k2t).view(np.uint8).reshape(E, MHDR)

    fvw = np.zeros((F + 1, NPAD + E), ml_dtypes.float8_e4m3fn)
    fvw[F, NPAD:] = np.asarray(b1).astype(ml_dtypes.float8_e4m3fn)
    fvw[0:F, NPAD:] = np.asarray(W1).astype(ml_dtypes.float8_e4m3fn)

    in_maps = []
    for d in range(NC):
        sl = slice(d * NPC, (d + 1) * NPC)
        fv = fvw.copy()
        fv[0:F, 0:NPC] = feature_vecs[sl].T.astype(ml_dtypes.float8_e4m3fn)
        fv[F, 0:NPC] = 1.0
        mt = np.zeros((NPAD, S), np.uint8)
        mt[:NPC] = m8[:, sl].T
        idx = mt[:HALF] + 2 * mt[HALF:]
        pk = lut[idx]
        pk4 = np.ascontiguousarray(
            pk.reshape(NSC, 2, 128, S).transpose(2, 0, 1, 3))
        flat = np.empty((128, MHDR + NSC * 2 * S), np.uint8)
        flat[:, 0:MHDR] = k2t_bytes
        flat[:, MHDR:] = pk4.reshape(128, NSC * 2 * S)
        in_maps.append({
            "fv": fv.view(ml_dtypes.float8_e4m3),
            "maskT": flat.view(ml_dtypes.float8_e4m3),
        })
    return in_maps


def kernel(feature_vecs, W1, b1, W2, b2, keys, rewards, mask, queue_idx, sel_idx):
    import sys
    if "/opt/trn_rl_repo" not in sys.path:
        sys.path.insert(0, "/opt/trn_rl_repo")
    from concourse.bass_utils import run_bass_kernel_spmd

    nc = _get_prog()
    in_maps = _prep(feature_vecs, W1, b1, W2, b2, keys, mask)
    res = run_bass_kernel_spmd(nc, in_maps, list(range(NC))).results

    qs = np.asarray(queue_idx).astype(np.int64)
    ar = np.arange(S)
    Z = np.zeros(S, np.float64)
    S1 = np.zeros(S, np.float64)
    cnt = np.asarray(mask).sum(axis=1, dtype=np.float64)
    for d in range(NC):
        raw = np.ascontiguousarray(res[d]["stats"])   # [128, 2050] f16
        st128 = raw[:, 0:S].astype(np.float64)
        anch = raw[:, S:S + 2].copy().view(np.float32).astype(np.float64)[:, 0]
        st = st128[0:64] + st128[64:128]
        a = anch[0:64] + anch[64:128]                 # [64] col sums
        D = st.copy()
        D[0:32] -= (a[0:32] * F00)[:, None]
        D[32:64] -= (a[32:64] * G00)[:, None]
        E1 = D[qs, ar] + D[8 + qs, ar]
        G1 = D[16 + qs, ar] + D[24 + qs, ar]
        E2 = D[32 + qs, ar] + D[40 + qs, ar]
        G2 = D[48 + qs, ar] + D[56 + qs, ar]
        Z += -SA * E1 + SB_ * E2
        S1 += 4.0 * (-SA * G1 + SB_ * G2)

    xall = np.stack([res[d]["xall"] for d in range(NC)]).astype(np.float64)
    sel = np.asarray(sel_idx).astype(np.int64)
    d_arr = sel // NPC
    nloc = sel % NPC
    x_sel = xall[d_arr, nloc % 128, nloc // 128, qs]

    logZ = np.log(Z)
    ce = logZ - x_sel
    me = (S1 / Z - logZ) / np.log(cnt)
    loss = (np.asarray(rewards, np.float64) * ce).sum() + ENTROPY_COEF * me.sum()
    return np.array([loss], dtype=np.float32)


# revision 24
# speedup vs baseline: 1.4235x; 1.4235x over previous
import math
import numpy as np
import ml_dtypes

N = 50000
F = 64
E = 128
Q = 8
S = 2048
NC = 8
NPC = N // NC          # 6250 clauses per core
NPAD = 6656            # 52 * 128 = 2 * 3328
HALF = 3328            # pair row count: clause nu pairs with nu + HALF
NSC = 13               # super-chunks of 2 128-row chunks
NCH = 52               # 128-clause chunks
SBK = 4                # psum banks of 512 steps each
MHDR = 32              # k2t (f32 bytes) header columns of the mask tensor
ENTROPY_COEF = 0.1

# 2-bit mask packing: byte encodes (m0, m1) of a clause pair; the same byte
# is read twice by the PE, once as fp8e4m3 (value f) and once as fp8e5m2
# (value g).  The quad has zero SECOND difference in both views:
#   f11-f10-f01+f00 = 0  and  g11-g10-g01+g00 = 0
# so two matmul passes recover m0*e0 + m1*e1 exactly; the anchor constant
# (state 00) is cancelled on the host using column sums of the fp8
# stationary tiles, which are DMA'd out.
B00, B10, B01, B11 = 0x0C, 0x17, 0x97, 0x8C
_by = np.array([B00, B10, B01, B11], np.uint8)
_f = _by.view(ml_dtypes.float8_e4m3fn).astype(np.float64)
_g = _by.view(ml_dtypes.float8_e5m2).astype(np.float64)
F00, G00 = _f[0], _g[0]
F10, G10 = _f[1] - _f[0], _g[1] - _g[0]
F01, G01 = _f[2] - _f[0], _g[2] - _g[0]
DET = F10 * G01 - F01 * G10
SA = 32.0              # pass-1 stationary scale (host multiplies by -SA)
SB_ = 1024.0           # pass-2 stationary scale (host multiplies by +SB_)
P1 = -G01 / (DET * SA)   # st1 = P1*e0 + P2*e1      (= 32/27, 8/9)
P2 = G10 / (DET * SA)
Q1 = -F01 / (DET * SB_)  # st2 = Q1*e0 + Q2*e1      (= 14/9, 2/3)
Q2 = F10 / (DET * SB_)

_PROG = None


def _build_prog():
    import sys
    if "/opt/trn_rl_repo" not in sys.path:
        sys.path.insert(0, "/opt/trn_rl_repo")
    from concourse import bass, bacc, tile, mybir

    f32 = mybir.dt.float32
    f16 = mybir.dt.float16
    bf16 = mybir.dt.bfloat16
    f8 = mybir.dt.float8e4
    f8e5 = mybir.dt.float8e5
    AF = mybir.ActivationFunctionType
    ALU = mybir.AluOpType

    nc = bacc.Bacc("TRN2")
    # fv carries [fvT | W1^T] fp8 with a 65th row of [ones | b1]
    fv_d = nc.dram_tensor("fv", [F + 1, NPAD + E], f8, kind="ExternalInput")
    # mask tensor: 32 header cols hold K2T as raw f32 bytes, then the packed
    # mask flattened as [13][2][2048]
    maskT_d = nc.dram_tensor("maskT", [128, MHDR + NSC * 2 * S], f8,
                             kind="ExternalInput")
    stats_d = nc.dram_tensor("stats", [128, SBK * 512 + 2], f16,
                             kind="ExternalOutput")
    xall_d = nc.dram_tensor("xall", [E, NCH, Q], f32, kind="ExternalOutput")

    with tile.TileContext(nc) as tc:
        with (
            tc.tile_pool(name="const", bufs=1) as constp,
            tc.tile_pool(name="big", bufs=1) as bigp,
            tc.tile_pool(name="mask", bufs=1) as maskp,
            tc.tile_pool(name="ps", bufs=1, space=bass.MemorySpace.PSUM) as ps,
        ):
            k2t_sb = constp.tile([E, Q], bf16)
            fsta = constp.tile([E, 1], f8)
            fmov = constp.tile([E, 512], f8)
            bias4 = constp.tile([E, 4], f32)

            fv_sb = bigp.tile([F + 1, NPAD + E], f8)
            ht_sb = bigp.tile([E, NPAD], bf16)
            xall_sb = bigp.tile([E, NCH, Q], f32)
            xs_sb = bigp.tile([E, NCH, Q], f32)
            u1 = bigp.tile([E, 26, Q], f32)
            v1 = bigp.tile([E, 26, Q], f32)
            u2 = bigp.tile([E, 26, Q], f32)
            v2 = bigp.tile([E, 26, Q], f32)
            aE = bigp.tile([E, 26, Q], f32)
            bE = bigp.tile([E, 26, Q], f32)
            aG = bigp.tile([E, 26, Q], f32)
            bG = bigp.tile([E, 26, Q], f32)
            t1 = bigp.tile([E, 26, Q], f32)
            t2 = bigp.tile([E, 26, Q], f32)
            t3 = bigp.tile([E, 26, Q], f32)
            hib = bigp.tile([E, 26, Q], f32)
            st12 = bigp.tile([E, 26, 64], f8)
            stats_sb = bigp.tile([128, SBK * 512 + 2], f16)

            w1_sb = fv_sb[:, NPAD:NPAD + E]

            LP1, LP2 = math.log(P1), math.log(P2)
            LQ1, LQ2 = math.log(Q1), math.log(Q2)

            # constants (vector engine, no deps)
            nc.vector.memset(fsta[:], 1.0)
            nc.vector.memset(fmov[:], 0.0)
            for bi, bv in enumerate((LP1, LP2, LQ1, LQ2)):
                nc.vector.memset(bias4[:, bi:bi + 1], bv)

            nc.sync.dma_start(fv_sb[:], fv_d[:])

            # mask stream on the SP ring in ~1MB blocks (block 0 carries the
            # k2t header)
            MBLK = [(2 * i, 2) for i in range(6)] + [(12, 1)]
            mts = {}
            for bi_, (s0, w) in enumerate(MBLK):
                hdr = MHDR if bi_ == 0 else 0
                mt = maskp.tile([128, hdr + w * 2 * S], f8, tag=f"m{s0}")
                c0 = MHDR + s0 * 2 * S - hdr
                nc.sync.dma_start(mt[:], maskT_d[:, c0:c0 + hdr + w * 2 * S])
                mts[s0] = (w, hdr, mt)

            def mslice(j, i, b):
                for s0, (w, hdr, mt) in mts.items():
                    if s0 <= j < s0 + w:
                        c = hdr + (j - s0) * 2 * S + i * S + 512 * b
                        return mt[:, c:c + 512]

            k2t_f32 = mts[0][2][:, 0:MHDR].bitcast(f32)

            pstat = ps.tile([128, SBK, 512], f32, tag="s", bufs=1, name="s")
            psr = ps.tile([128, 1], f32, tag="r", bufs=1, name="r")
            xps = ps.tile([E, NCH, Q], f32, tag="x", bufs=1, name="x")

            # PE warm-up fillers with K=128 so the HAM activity monitor sees
            # real array utilization and unthrottles before the first matmul.
            for _ in range(9):
                nc.tensor.matmul(pstat[0:1, 0, :], fsta[:], fmov[:],
                                 start=True, stop=True)

            # hT = relu(W1.T @ fvT + b1)  [E, NPAD] bf16; each relu is
            # split in half across ACT and DVE so it never paces the matmuls
            for j in range(NPAD // 512):
                c0 = 512 * j
                ph = ps.tile([E, 512], f32, tag="w", bufs=2, name="w")
                nc.tensor.matmul(ph[:], w1_sb, fv_sb[:, c0:c0 + 512],
                                 start=True, stop=True)
                nc.scalar.activation(ht_sb[:, c0:c0 + 256], ph[:, 0:256],
                                     AF.Relu, bias=0.0, scale=1.0)
                nc.vector.tensor_scalar(ht_sb[:, c0 + 256:c0 + 512],
                                        ph[:, 256:512], 0.0, None, ALU.max)

            # k2t: bf16 copy of the f32 header (after the relus on ACT so it
            # doesn't head-block them while mask block 0 is in flight)
            nc.scalar.activation(k2t_sb[:], k2t_f32, AF.Copy)

            # x'[n,q] = hT_chunk.T @ K2T, pair-interleaved chunk order so the
            # prep of super-chunk k only needs the first 4(k+1) x' matmuls
            for k in range(NSC):
                for i in range(2):
                    c = 2 * k + i
                    nc.tensor.matmul(xps[:, c, :], ht_sb[:, 128 * c:128 * (c + 1)],
                                     k2t_sb[:], start=True, stop=True)
                    c2 = c + 26
                    nc.tensor.matmul(xps[:, c2, :], ht_sb[:, 128 * c2:128 * (c2 + 1)],
                                     k2t_sb[:], start=True, stop=True)

            # bridge fillers: PE stays busy while block-0 prep runs on ACT/DVE
            for _ in range(3):
                nc.tensor.matmul(pstat[0:1, 0, :], fsta[:], fmov[:],
                                 start=True, stop=True)

            first = True
            for (k0, k1) in [(0, 0), (1, 2), (3, 4), (5, 6), (7, 8), (9, 10), (11, 12)]:
                kk = slice(2 * k0, 2 * k1 + 2)
                kx = slice(2 * k0 + 26, 2 * k1 + 2 + 26)
                # x/4 pre-scale keeps every G-stationary fp8 value < 128:
                # values >= 256 (top e4m3 binade) break the mixed-dtype multiply
                nc.scalar.activation(xs_sb[:, kk, :], xps[:, kk, :], AF.Copy,
                                     scale=0.25)
                nc.scalar.activation(xs_sb[:, kx, :], xps[:, kx, :], AF.Copy,
                                     scale=0.25)
                # stationary values: st1 = P1*e0 + P2*e1 (pass1, scale -SA),
                #                    st2 = Q1*e0 + Q2*e1 (pass2, scale +SB_)
                nc.scalar.activation(u1[:, kk, :], xps[:, kk, :], AF.Exp,
                                     bias=bias4[:, 0:1])
                nc.scalar.activation(v1[:, kk, :], xps[:, kx, :], AF.Exp,
                                     bias=bias4[:, 1:2])
                nc.scalar.activation(u2[:, kk, :], xps[:, kk, :], AF.Exp,
                                     bias=bias4[:, 2:3])
                nc.scalar.activation(v2[:, kk, :], xps[:, kx, :], AF.Exp,
                                     bias=bias4[:, 3:4])
                nc.vector.tensor_tensor(aE[:, kk, :], u1[:, kk, :], v1[:, kk, :], ALU.add)
                nc.vector.tensor_tensor(bE[:, kk, :], u2[:, kk, :], v2[:, kk, :], ALU.add)
                nc.gpsimd.tensor_tensor(t1[:, kk, :], xs_sb[:, kk, :], u1[:, kk, :], ALU.mult)
                nc.gpsimd.tensor_tensor(t2[:, kk, :], xs_sb[:, kx, :], v1[:, kk, :], ALU.mult)
                nc.vector.tensor_tensor(aG[:, kk, :], t1[:, kk, :], t2[:, kk, :], ALU.add)
                nc.vector.tensor_tensor(t3[:, kk, :], xs_sb[:, kk, :], u2[:, kk, :], ALU.mult)
                nc.vector.tensor_tensor(t2[:, kk, :], xs_sb[:, kx, :], v2[:, kk, :], ALU.mult)
                nc.vector.tensor_tensor(bG[:, kk, :], t3[:, kk, :], t2[:, kk, :], ALU.add)
                # hi/lo fp8e4m3 splits: hi-cast on GpSimd, then the lo
                # residual via a direct mixed-dtype subtract on DVE
                for srct, cb in ((aE, 0), (aG, 16), (bE, 32), (bG, 48)):
                    dst = st12
                    nc.gpsimd.tensor_copy(dst[:, kk, cb:cb + 8], srct[:, kk, :])
                    nc.vector.tensor_tensor(dst[:, kk, cb + 8:cb + 16],
                                            srct[:, kk, :],
                                            dst[:, kk, cb:cb + 8],
                                            ALU.subtract)

                if first:
                    # x outputs are final after block 0's inputs; ship early
                    nc.scalar.activation(xall_sb[:], xps[:], AF.Copy)
                    nc.scalar.dma_start(xall_d[:], xall_sb[:])
                    first = False

                # stats matmuls: 4 concurrent column groups per (j, bank):
                #   cg0 = (i=0, pass1/e4)   cg1 = (i=0, pass2/e5)
                #   cg2 = (i=1, pass1/e4)   cg3 = (i=1, pass2/e5)
                for j in range(k0, k1 + 1):
                    start = (j == 0)
                    last = (j == NSC - 1)
                    for b in range(SBK):
                        for i in range(2):
                            mv = mslice(j, i, b)
                            s1k = st12[:, 2 * j + i, 0:32]
                            s2k = st12[:, 2 * j + i, 32:64]
                            cg = 64 * i
                            nc.tensor.matmul(pstat[cg:cg + 32, b, :], s1k, mv,
                                             start=start, stop=last,
                                             skip_group_check=True,
                                             tile_position=(0, cg))
                            nc.tensor.matmul(pstat[cg + 32:cg + 64, b, :], s2k,
                                             mv.bitcast(f8e5),
                                             start=start, stop=last,
                                             skip_group_check=True,
                                             tile_position=(0, cg + 32))
                    # anchor columns: per-column stationary sums via a
                    # ones-column matmul over the merged 64-col tile
                    for i in range(2):
                        cg = 64 * i
                        nc.tensor.matmul(psr[cg:cg + 64, :],
                                         st12[:, 2 * j + i, :], fsta[:],
                                         start=start, stop=last,
                                         skip_group_check=True,
                                         tile_position=(0, cg))
                    if last:
                        nc.scalar.activation(stats_sb[:, 0:512],
                                             pstat[:, 0, :], AF.Copy)
                        nc.vector.tensor_copy(stats_sb[:, 512:1024],
                                              pstat[:, 1, :])
                        nc.sync.dma_start(stats_d[:, 0:1024],
                                          stats_sb[:, 0:1024])
                        nc.scalar.activation(stats_sb[:, 1024:1536],
                                             pstat[:, 2, :], AF.Copy)
                        nc.vector.tensor_copy(stats_sb[:, 1536:2048],
                                              pstat[:, 3, :])
                        nc.vector.tensor_copy(
                            stats_sb[:, 2048:2050].bitcast(f32), psr[:])
                        nc.sync.dma_start(stats_d[:, 1024:2050],
                                          stats_sb[:, 1024:2050])

    nc.finalize()
    return nc


def _get_prog():
    global _PROG
    if _PROG is None:
        _PROG = _build_prog()
    return _PROG


def _prep(feature_vecs, W1, b1, W2, b2, keys, mask):
    m8 = mask.view(np.uint8) if mask.dtype == np.bool_ else mask.astype(np.uint8)
    lut = np.array([B00, B10, B01, B11], np.uint8)

    k2t = (np.asarray(W2, np.float64) @ np.asarray(keys, np.float64).T
           ).astype(np.float32)                       # [E, Q]
    k2t_bytes = np.ascontiguousarray(k2t).view(np.uint8).reshape(E, MHDR)

    fvw = np.zeros((F + 1, NPAD + E), ml_dtypes.float8_e4m3fn)
    fvw[F, NPAD:] = np.asarray(b1).astype(ml_dtypes.float8_e4m3fn)
    fvw[0:F, NPAD:] = np.asarray(W1).astype(ml_dtypes.float8_e4m3fn)

    in_maps = []
    for d in range(NC):
        sl = slice(d * NPC, (d + 1) * NPC)
        fv = fvw.copy()
        fv[0:F, 0:NPC] = feature_vecs[sl].T.astype(ml_dtypes.float8_e4m3fn)
        fv[F, 0:NPC] = 1.0
        mt = np.zeros((NPAD, S), np.uint8)
        mt[:NPC] = m8[:, sl].T
        idx = mt[:HALF] + 2 * mt[HALF:]
        pk = lut[idx]
        pk4 = np.ascontiguousarray(
            pk.reshape(NSC, 2, 128, S).transpose(2, 0, 1, 3))
        flat = np.empty((128, MHDR + NSC * 2 * S), np.uint8)
        flat[:, 0:MHDR] = k2t_bytes
        flat[:, MHDR:] = pk4.reshape(128, NSC * 2 * S)
        in_maps.append({
            "fv": fv.view(ml_dtypes.float8_e4m3),
            "maskT": flat.view(ml_dtypes.float8_e4m3),
        })
    return in_maps


def kernel(feature_vecs, W1, b1, W2, b2, keys, rewards, mask, queue_idx, sel_idx):
    import sys
    if "/opt/trn_rl_repo" not in sys.path:
        sys.path.insert(0, "/opt/trn_rl_repo")
    from concourse.bass_utils import run_bass_kernel_spmd

    nc = _get_prog()
    in_maps = _prep(feature_vecs, W1, b1, W2, b2, keys, mask)
    res = run_bass_kernel_spmd(nc, in_maps, list(range(NC))).results

    qs = np.asarray(queue_idx).astype(np.int64)
    ar = np.arange(S)
    Z = np.zeros(S, np.float64)
    S1 = np.zeros(S, np.float64)
    cnt = np.asarray(mask).sum(axis=1, dtype=np.float64)
    for d in range(NC):
        raw = np.ascontiguousarray(res[d]["stats"])   # [128, 2050] f16
        st128 = raw[:, 0:S].astype(np.float64)
        anch = raw[:, S:S + 2].copy().view(np.float32).astype(np.float64)[:, 0]
        st = st128[0:64] + st128[64:128]
        a = anch[0:64] + anch[64:128]                 # [64] col sums
        D = st.copy()
        D[0:32] -= (a[0:32] * F00)[:, None]
        D[32:64] -= (a[32:64] * G00)[:, None]
        E1 = D[qs, ar] + D[8 + qs, ar]
        G1 = D[16 + qs, ar] + D[24 + qs, ar]
        E2 = D[32 + qs, ar] + D[40 + qs, ar]
        G2 = D[48 + qs, ar] + D[56 + qs, ar]
        Z += -SA * E1 + SB_ * E2
        S1 += 4.0 * (-SA * G1 + SB_ * G2)

    xall = np.stack([res[d]["xall"] for d in range(NC)]).astype(np.float64)
    sel = np.asarray(sel_idx).astype(np.int64)
    d_arr = sel // NPC
    nloc = sel % NPC
    x_sel = xall[d_arr, nloc % 128, nloc // 128, qs]

    logZ = np.log(Z)
    ce = logZ - x_sel
    me = (S1 / Z - logZ) / np.log(cnt)
    loss = (np.asarray(rewards, np.float64) * ce).sum() + ENTROPY_COEF * me.sum()
    return np.array([loss], dtype=np.float32)


# revision 25
# speedup vs baseline: 1.6114x; 1.1320x over previous
import math
import numpy as np
import ml_dtypes

N = 50000
F = 64
E = 128
Q = 8
S = 2048
NC = 8
NPC = N // NC          # 6250 clauses per core
NPAD = 6656            # 52 * 128 = 2 * 3328
HALF = 3328            # pair row count: clause nu pairs with nu + HALF
NSC = 13               # super-chunks of 2 128-row chunks
NCH = 52               # 128-clause chunks
SBK = 4                # psum banks of 512 steps each
MHDR = 32              # k2t (f32 bytes) header columns of the mask tensor
ENTROPY_COEF = 0.1

# 2-bit mask packing: byte encodes (m0, m1) of a clause pair; the same byte
# is read twice by the PE, once as fp8e4m3 (value f) and once as fp8e5m2
# (value g).  The quad has zero SECOND difference in both views:
#   f11-f10-f01+f00 = 0  and  g11-g10-g01+g00 = 0
# so two matmul passes recover m0*e0 + m1*e1 exactly; the anchor constant
# (state 00) is cancelled on the host using column sums of the fp8
# stationary tiles, which are DMA'd out.
B00, B10, B01, B11 = 0x0C, 0x17, 0x97, 0x8C
_by = np.array([B00, B10, B01, B11], np.uint8)
_f = _by.view(ml_dtypes.float8_e4m3fn).astype(np.float64)
_g = _by.view(ml_dtypes.float8_e5m2).astype(np.float64)
F00, G00 = _f[0], _g[0]
F10, G10 = _f[1] - _f[0], _g[1] - _g[0]
F01, G01 = _f[2] - _f[0], _g[2] - _g[0]
DET = F10 * G01 - F01 * G10
SA = 32.0              # pass-1 stationary scale (host multiplies by -SA)
SB_ = 1024.0           # pass-2 stationary scale (host multiplies by +SB_)
P1 = -G01 / (DET * SA)   # st1 = P1*e0 + P2*e1      (= 32/27, 8/9)
P2 = G10 / (DET * SA)
Q1 = -F01 / (DET * SB_)  # st2 = Q1*e0 + Q2*e1      (= 14/9, 2/3)
Q2 = F10 / (DET * SB_)

_PROG = None


def _build_prog():
    import sys
    if "/opt/trn_rl_repo" not in sys.path:
        sys.path.insert(0, "/opt/trn_rl_repo")
    from concourse import bass, bacc, tile, mybir

    f32 = mybir.dt.float32
    f16 = mybir.dt.float16
    bf16 = mybir.dt.bfloat16
    f8 = mybir.dt.float8e4
    f8e5 = mybir.dt.float8e5
    AF = mybir.ActivationFunctionType
    ALU = mybir.AluOpType

    nc = bacc.Bacc("TRN2")
    # fv carries [fvT | W1^T] fp8 with a 65th row of [ones | b1]
    fv_d = nc.dram_tensor("fv", [F + 1, NPAD + E], f8, kind="ExternalInput")
    # mask tensor: 32 header cols hold K2T as raw f32 bytes, then the packed
    # mask flattened as [13][2][2048]
    maskT_d = nc.dram_tensor("maskT", [128, MHDR + NSC * 2 * S], f8,
                             kind="ExternalInput")
    stats_d = nc.dram_tensor("stats", [128, SBK * 512], f16,
                             kind="ExternalOutput")
    st12_d = nc.dram_tensor("st12", [E, 26, 64], f8, kind="ExternalOutput")
    xall_d = nc.dram_tensor("xall", [E, NCH, Q], f32, kind="ExternalOutput")

    with tile.TileContext(nc) as tc:
        with (
            tc.tile_pool(name="const", bufs=1) as constp,
            tc.tile_pool(name="big", bufs=1) as bigp,
            tc.tile_pool(name="mask", bufs=1) as maskp,
            tc.tile_pool(name="ps", bufs=1, space=bass.MemorySpace.PSUM) as ps,
        ):
            k2t_sb = constp.tile([E, Q], bf16)
            fsta = constp.tile([E, 1], f8)
            fmov = constp.tile([E, 512], f8)
            bias4 = constp.tile([E, 4], f32)

            fv_sb = bigp.tile([F + 1, NPAD + E], f8)
            ht_sb = bigp.tile([E, NPAD], bf16)
            xall_sb = bigp.tile([E, NCH, Q], f32)
            xs_sb = bigp.tile([E, NCH, Q], f32)
            u1 = bigp.tile([E, 26, Q], f32)
            v1 = bigp.tile([E, 26, Q], f32)
            u2 = bigp.tile([E, 26, Q], f32)
            v2 = bigp.tile([E, 26, Q], f32)
            aE = bigp.tile([E, 26, Q], f32)
            bE = bigp.tile([E, 26, Q], f32)
            aG = bigp.tile([E, 26, Q], f32)
            bG = bigp.tile([E, 26, Q], f32)
            t1 = bigp.tile([E, 26, Q], f32)
            t2 = bigp.tile([E, 26, Q], f32)
            t3 = bigp.tile([E, 26, Q], f32)
            hib = bigp.tile([E, 26, Q], f32)
            st12 = bigp.tile([E, 26, 64], f8)
            stats_sb = bigp.tile([128, SBK * 512], f16)

            w1_sb = fv_sb[:, NPAD:NPAD + E]

            LP1, LP2 = math.log(P1), math.log(P2)
            LQ1, LQ2 = math.log(Q1), math.log(Q2)

            # constants (vector engine, no deps)
            nc.vector.memset(fsta[:], 1.0)
            nc.vector.memset(fmov[:], 0.0)
            for bi, bv in enumerate((LP1, LP2, LQ1, LQ2)):
                nc.vector.memset(bias4[:, bi:bi + 1], bv)

            nc.sync.dma_start(fv_sb[:], fv_d[:])

            # mask stream on the SP ring in ~1MB blocks (block 0 carries the
            # k2t header)
            MBLK = [(2 * i, 2) for i in range(6)] + [(12, 1)]
            mts = {}
            for bi_, (s0, w) in enumerate(MBLK):
                hdr = MHDR if bi_ == 0 else 0
                mt = maskp.tile([128, hdr + w * 2 * S], f8, tag=f"m{s0}")
                c0 = MHDR + s0 * 2 * S - hdr
                nc.sync.dma_start(mt[:], maskT_d[:, c0:c0 + hdr + w * 2 * S])
                mts[s0] = (w, hdr, mt)

            def mslice(j, i, b):
                for s0, (w, hdr, mt) in mts.items():
                    if s0 <= j < s0 + w:
                        c = hdr + (j - s0) * 2 * S + i * S + 512 * b
                        return mt[:, c:c + 512]

            k2t_f32 = mts[0][2][:, 0:MHDR].bitcast(f32)

            pstat = ps.tile([128, SBK, 512], f32, tag="s", bufs=1, name="s")
            xps = ps.tile([E, NCH, Q], f32, tag="x", bufs=1, name="x")

            # PE warm-up fillers with K=128 so the HAM activity monitor sees
            # real array utilization and unthrottles before the first matmul.
            for _ in range(9):
                nc.tensor.matmul(pstat[0:1, 0, :], fsta[:], fmov[:],
                                 start=True, stop=True)

            # hT = relu(W1.T @ fvT + b1)  [E, NPAD] bf16; each relu is
            # split in half across ACT and DVE so it never paces the matmuls
            for j in range(NPAD // 512):
                c0 = 512 * j
                ph = ps.tile([E, 512], f32, tag="w", bufs=3, name="w")
                nc.tensor.matmul(ph[:], w1_sb, fv_sb[:, c0:c0 + 512],
                                 start=True, stop=True)
                nc.scalar.activation(ht_sb[:, c0:c0 + 256], ph[:, 0:256],
                                     AF.Relu, bias=0.0, scale=1.0)
                nc.vector.tensor_scalar(ht_sb[:, c0 + 256:c0 + 512],
                                        ph[:, 256:512], 0.0, None, ALU.max)
                if j % 2 == 1:
                    nc.tensor.matmul(pstat[0:1, 0, :], fsta[:], fmov[:],
                                     start=True, stop=True)

            # k2t: bf16 copy of the f32 header (after the relus on ACT so it
            # doesn't head-block them while mask block 0 is in flight)
            nc.scalar.activation(k2t_sb[:], k2t_f32, AF.Copy)

            # x'[n,q] = hT_chunk.T @ K2T, pair-interleaved chunk order so the
            # prep of super-chunk k only needs the first 4(k+1) x' matmuls
            for k in range(NSC):
                for i in range(2):
                    c = 2 * k + i
                    nc.tensor.matmul(xps[:, c, :], ht_sb[:, 128 * c:128 * (c + 1)],
                                     k2t_sb[:], start=True, stop=True)
                    c2 = c + 26
                    nc.tensor.matmul(xps[:, c2, :], ht_sb[:, 128 * c2:128 * (c2 + 1)],
                                     k2t_sb[:], start=True, stop=True)

            # bridge fillers: PE stays busy while block-0 prep runs on ACT/DVE
            for _ in range(3):
                nc.tensor.matmul(pstat[0:1, 0, :], fsta[:], fmov[:],
                                 start=True, stop=True)

            first = True
            for (k0, k1) in [(0, 0), (1, 2), (3, 4), (5, 6), (7, 8), (9, 10), (11, 12)]:
                kk = slice(2 * k0, 2 * k1 + 2)
                kx = slice(2 * k0 + 26, 2 * k1 + 2 + 26)
                # x/4 pre-scale keeps every G-stationary fp8 value < 128:
                # values >= 256 (top e4m3 binade) break the mixed-dtype multiply
                nc.scalar.activation(xs_sb[:, kk, :], xps[:, kk, :], AF.Copy,
                                     scale=0.25)
                nc.scalar.activation(xs_sb[:, kx, :], xps[:, kx, :], AF.Copy,
                                     scale=0.25)
                # stationary values: st1 = P1*e0 + P2*e1 (pass1, scale -SA),
                #                    st2 = Q1*e0 + Q2*e1 (pass2, scale +SB_)
                nc.scalar.activation(u1[:, kk, :], xps[:, kk, :], AF.Exp,
                                     bias=bias4[:, 0:1])
                nc.scalar.activation(v1[:, kk, :], xps[:, kx, :], AF.Exp,
                                     bias=bias4[:, 1:2])
                nc.scalar.activation(u2[:, kk, :], xps[:, kk, :], AF.Exp,
                                     bias=bias4[:, 2:3])
                nc.scalar.activation(v2[:, kk, :], xps[:, kx, :], AF.Exp,
                                     bias=bias4[:, 3:4])
                nc.vector.tensor_tensor(aE[:, kk, :], u1[:, kk, :], v1[:, kk, :], ALU.add)
                nc.vector.tensor_tensor(bE[:, kk, :], u2[:, kk, :], v2[:, kk, :], ALU.add)
                nc.gpsimd.tensor_tensor(t1[:, kk, :], xs_sb[:, kk, :], u1[:, kk, :], ALU.mult)
                nc.gpsimd.tensor_tensor(t2[:, kk, :], xs_sb[:, kx, :], v1[:, kk, :], ALU.mult)
                nc.vector.tensor_tensor(aG[:, kk, :], t1[:, kk, :], t2[:, kk, :], ALU.add)
                nc.vector.tensor_tensor(t3[:, kk, :], xs_sb[:, kk, :], u2[:, kk, :], ALU.mult)
                nc.vector.tensor_tensor(t2[:, kk, :], xs_sb[:, kx, :], v2[:, kk, :], ALU.mult)
                nc.vector.tensor_tensor(bG[:, kk, :], t3[:, kk, :], t2[:, kk, :], ALU.add)
                # hi/lo fp8e4m3 splits: hi-cast on GpSimd, then the lo
                # residual via a direct mixed-dtype subtract on DVE
                for srct, cb in ((aE, 0), (aG, 16), (bE, 32), (bG, 48)):
                    dst = st12
                    nc.gpsimd.tensor_copy(dst[:, kk, cb:cb + 8], srct[:, kk, :])
                    nc.vector.tensor_tensor(dst[:, kk, cb + 8:cb + 16],
                                            srct[:, kk, :],
                                            dst[:, kk, cb:cb + 8],
                                            ALU.subtract)

                if first:
                    # x outputs are final after block 0's inputs; ship early
                    nc.scalar.activation(xall_sb[:], xps[:], AF.Copy)
                    nc.scalar.dma_start(xall_d[:], xall_sb[:])
                    first = False

                # stats matmuls: 4 concurrent column groups per (j, bank):
                #   cg0 = (i=0, pass1/e4)   cg1 = (i=0, pass2/e5)
                #   cg2 = (i=1, pass1/e4)   cg3 = (i=1, pass2/e5)
                for j in range(k0, k1 + 1):
                    start = (j == 0)
                    last = (j == NSC - 1)
                    for b in range(SBK):
                        for i in range(2):
                            mv = mslice(j, i, b)
                            s1k = st12[:, 2 * j + i, 0:32]
                            s2k = st12[:, 2 * j + i, 32:64]
                            cg = 64 * i
                            nc.tensor.matmul(pstat[cg:cg + 32, b, :], s1k, mv,
                                             start=start, stop=last,
                                             skip_group_check=True,
                                             tile_position=(0, cg))
                            nc.tensor.matmul(pstat[cg + 32:cg + 64, b, :], s2k,
                                             mv.bitcast(f8e5),
                                             start=start, stop=last,
                                             skip_group_check=True,
                                             tile_position=(0, cg + 32))

                    if last:
                        nc.scalar.activation(stats_sb[:, 0:512],
                                             pstat[:, 0, :], AF.Copy)
                        nc.vector.tensor_copy(stats_sb[:, 512:1024],
                                              pstat[:, 1, :])
                        nc.sync.dma_start(stats_d[:, 0:1024],
                                          stats_sb[:, 0:1024])
                        nc.scalar.activation(stats_sb[:, 1024:1536],
                                             pstat[:, 2, :], AF.Copy)
                        nc.vector.tensor_copy(stats_sb[:, 1536:2048],
                                              pstat[:, 3, :])
                        nc.sync.dma_start(stats_d[:, 1024:2048],
                                          stats_sb[:, 1024:2048])
                        nc.sync.dma_start(st12_d[:], st12[:])

    nc.finalize()
    return nc


def _get_prog():
    global _PROG
    if _PROG is None:
        _PROG = _build_prog()
    return _PROG


def _prep(feature_vecs, W1, b1, W2, b2, keys, mask):
    m8 = mask.view(np.uint8) if mask.dtype == np.bool_ else mask.astype(np.uint8)
    lut = np.array([B00, B10, B01, B11], np.uint8)

    k2t = (np.asarray(W2, np.float64) @ np.asarray(keys, np.float64).T
           ).astype(np.float32)                       # [E, Q]
    k2t_bytes = np.ascontiguousarray(k2t).view(np.uint8).reshape(E, MHDR)

    fvw = np.zeros((F + 1, NPAD + E), ml_dtypes.float8_e4m3fn)
    fvw[F, NPAD:] = np.asarray(b1).astype(ml_dtypes.float8_e4m3fn)
    fvw[0:F, NPAD:] = np.asarray(W1).astype(ml_dtypes.float8_e4m3fn)

    in_maps = []
    for d in range(NC):
        sl = slice(d * NPC, (d + 1) * NPC)
        fv = fvw.copy()
        fv[0:F, 0:NPC] = feature_vecs[sl].T.astype(ml_dtypes.float8_e4m3fn)
        fv[F, 0:NPC] = 1.0
        mt = np.zeros((NPAD, S), np.uint8)
        mt[:NPC] = m8[:, sl].T
        idx = mt[:HALF] + 2 * mt[HALF:]
        pk = lut[idx]
        pk4 = np.ascontiguousarray(
            pk.reshape(NSC, 2, 128, S).transpose(2, 0, 1, 3))
        flat = np.empty((128, MHDR + NSC * 2 * S), np.uint8)
        flat[:, 0:MHDR] = k2t_bytes
        flat[:, MHDR:] = pk4.reshape(128, NSC * 2 * S)
        in_maps.append({
            "fv": fv.view(ml_dtypes.float8_e4m3),
            "maskT": flat.view(ml_dtypes.float8_e4m3),
        })
    return in_maps


def kernel(feature_vecs, W1, b1, W2, b2, keys, rewards, mask, queue_idx, sel_idx):
    import sys
    if "/opt/trn_rl_repo" not in sys.path:
        sys.path.insert(0, "/opt/trn_rl_repo")
    from concourse.bass_utils import run_bass_kernel_spmd

    nc = _get_prog()
    in_maps = _prep(feature_vecs, W1, b1, W2, b2, keys, mask)
    res = run_bass_kernel_spmd(nc, in_maps, list(range(NC))).results

    qs = np.asarray(queue_idx).astype(np.int64)
    ar = np.arange(S)
    Z = np.zeros(S, np.float64)
    S1 = np.zeros(S, np.float64)
    cnt = np.asarray(mask).sum(axis=1, dtype=np.float64)
    for d in range(NC):
        st128 = res[d]["stats"].astype(np.float64)    # [128, 2048]
        s12 = np.asarray(res[d]["st12"]).view(ml_dtypes.float8_e4m3fn
                                              ).astype(np.float64)  # [128,26,64]
        a_ev = s12[:, 0::2, :].sum(axis=(0, 1))       # [64]
        a_od = s12[:, 1::2, :].sum(axis=(0, 1))
        fg = np.concatenate([np.full(32, F00), np.full(32, G00)])
        D128 = st128.copy()
        D128[0:64] -= (a_ev * fg)[:, None]
        D128[64:128] -= (a_od * fg)[:, None]
        D = D128[0:64] + D128[64:128]
        E1 = D[qs, ar] + D[8 + qs, ar]
        G1 = D[16 + qs, ar] + D[24 + qs, ar]
        E2 = D[32 + qs, ar] + D[40 + qs, ar]
        G2 = D[48 + qs, ar] + D[56 + qs, ar]
        Z += -SA * E1 + SB_ * E2
        S1 += 4.0 * (-SA * G1 + SB_ * G2)

    xall = np.stack([res[d]["xall"] for d in range(NC)]).astype(np.float64)
    sel = np.asarray(sel_idx).astype(np.int64)
    d_arr = sel // NPC
    nloc = sel % NPC
    x_sel = xall[d_arr, nloc % 128, nloc // 128, qs]

    logZ = np.log(Z)
    ce = logZ - x_sel
    me = (S1 / Z - logZ) / np.log(cnt)
    loss = (np.asarray(rewards, np.float64) * ce).sum() + ENTROPY_COEF * me.sum()
    return np.array([loss], dtype=np.float32)
